# revision 17
# baseline (speedup 1.0000x reference)
"""Trainium2 Bass kernel for a 2-layer directed GraphSAGE (DirectedGNN).

Computation (matching the reference):
    w = sigmoid(edge_weight); src, dst = edge_index
    s1 = relu(mean_{e: dst=i} w_e * t[src_e] @ s0_Wl.T + s0_bl + t @ s0_Wr.T)
    t1 = relu(mean_{e: src=i} w_e * s[dst_e] @ t0_Wl.T + t0_bl + s @ t0_Wr.T)
    s2 =      mean_{e: dst=i} w_e * t1[src_e] @ s1_Wl.T + s1_bl + t1 @ s1_Wr.T
    t2 =      mean_{e: src=i} w_e * s1[dst_e] @ t1_Wl.T + t1_bl + s1 @ t1_Wr.T
    returns (s2, t2)

Strategy (8 NeuronCores, node-parallel with device-side replication):
  * Edges sorted by aggregation node (dst for s-updates, src for t-updates);
    nodes sharded contiguously across the 8 cores, so every core's segment
    sums are complete locally (no all-reduce).
  * The wall clock is dominated by the axon tunnel (~30-40 MB/s), so wire
    bytes are minimized: each core receives only its own node-feature shard,
    quantized to int8 (the quantization scale is folded into the layer-0
    weights on the host, so the device runs entirely in scaled units with
    no dequant step).  Full fp16 gather tables are built on device with an
    AllGather.  Index streams ship 16 partitions wide / rel indices as int8
    and are widened on device.  All params are consolidated into a handful
    of arrays (device_put dispatch overhead), and both outputs leave as one
    fp16 array.
  * Aggregation on TensorE: for each 128-node window, edges are processed
    in chunks of 128 (one per SBUF partition).  Gathered neighbor features
    (fp16, via indirect DMA) are the stationary operand; a one-hot
    selection matrix S[e, n] = w'_e * (dst_rel_e == n) built on VectorE is
    the moving operand.  PSUM accumulates mean^T directly (w' pre-scaled
    by 1/deg on the host).  Row-orientation copies of layer-0 outputs (the
    next layer's gather tables) come from PE transposes of the column
    outputs rather than a second GEMM pair.
  * The runner caches the jitted shard_map closure across calls and
    overlaps async device_put uploads with the host-side edge sort.
"""

import numpy as np

import sys

sys.path.insert(0, "/opt/trn_rl_repo")

import concourse.bass as bass  # noqa: E402
import concourse.bacc as bacc  # noqa: E402
import concourse.mybir as mybir  # noqa: E402
import concourse.tile as tile  # noqa: E402

P = 128  # partitions / feature dim / node window
D = 128

F32 = mybir.dt.float32
F16 = mybir.dt.float16
I32 = mybir.dt.int32
I16 = mybir.dt.int16
I8 = mybir.dt.int8


# ---------------------------------------------------------------------------
# Host-side preprocessing
# ---------------------------------------------------------------------------

HALF = 32768  # dma_gather int16 index limit -> split tables in two halves


def _prep_direction(agg, gat, w_eff, N, NC):
    """Sort edges by aggregation node, shard + window + chunk them.

    Within each 128-node window, edges are ordered [table-lo | table-hi]
    (dma_gather indices are int16, so the node table is gathered in two
    halves).  Both groups are padded to a chunk multiple; chunk counts
    (T_lo, T_hi) are global maxima so the program is SPMD-uniform.

    Edge order within a (window, half) group is irrelevant (they sum into
    the same PSUM accumulation), so a single stable argsort on the
    composite group key suffices.

    Returns (T_lo, T_hi, idx16, rel, wgt) as concat-ready global arrays:
      idx16 -- [NC*16, NW*T*8] int16  dma_gather index stream (16-partition
               wrap; replicated to 128 partitions on device)
      rel   -- [NC*P, NW*T] int8      agg node index relative to its window
      wgt   -- [NC*P, NW*T] f16       w * 1/deg(agg), 0 for padding slots
    Slot (p, w*T + c) holds edge c*128+p of window w.
    """
    SHARD = N // NC
    NW = -(-SHARD // P)
    SHARD_PAD = NW * P
    PAD_GAP = SHARD_PAD - SHARD

    core, off = np.divmod(agg, SHARD)
    win, rel = np.divmod(off, P)
    gw = core * NW + win
    gp = gat + PAD_GAP * (gat // SHARD)
    is_hi = (gp >= HALF).astype(np.int32)
    key = gw * 2 + is_hi

    order = np.argsort(key)  # within-group order irrelevant
    sub = key[order]
    gp = gp[order]
    rel = rel[order]
    ww = w_eff[order]
    hi_s = sub & 1

    cnt = np.bincount(sub, minlength=NC * NW * 2)
    cnt_lo, cnt_hi = cnt[0::2], cnt[1::2]
    T_lo = int(-(-cnt_lo.max() // P))
    T_hi = int(-(-cnt_hi.max() // P))
    T = T_lo + T_hi
    S = T * P

    starts = np.zeros(NC * NW * 2 + 1, np.int64)
    starts[1:] = np.cumsum(cnt)
    rank = np.arange(len(sub)) - starts[sub]
    slot = rank + hi_s * (T_lo * P)
    gww = sub >> 1

    idx16 = np.zeros((NC * NW, S), np.int16)
    relA = np.zeros((NC * NW, S), np.int8)
    wgtA = np.zeros((NC * NW, S), np.float16)
    idx16[gww, slot] = (gp - hi_s * HALF).astype(np.int16)
    relA[gww, slot] = rel.astype(np.int8)
    wgtA[gww, slot] = ww.astype(np.float16)

    def lay(x):
        # [NC*NW, T*P] -> [NC, NW, T, P] -> [NC, P, NW, T] -> [NC*P, NW*T]
        return np.ascontiguousarray(
            x.reshape(NC, NW, T, P).transpose(0, 3, 1, 2)
        ).reshape(NC * P, NW * T)

    # dma_gather idx stream: slot s -> partition s%16, column s//16.
    iw = idx16.reshape(NC, NW, T * 8, 16).transpose(0, 3, 1, 2)
    iw = np.ascontiguousarray(iw).reshape(NC * 16, NW * T * 8)

    return T_lo, T_hi, iw, lay(relA), lay(wgtA)


# ---------------------------------------------------------------------------
# Device program
# ---------------------------------------------------------------------------

def build_program(N, NC, Tlo_s, Thi_s, Tlo_t, Thi_t):
    T_s = Tlo_s + Thi_s
    T_t = Tlo_t + Thi_t
    NWT_s = None  # set below
    SHARD = N // NC
    NW = -(-SHARD // P)
    SHARD_PAD = NW * P
    N_PAD = NC * SHARD_PAD
    NWT_s = NW * T_s
    NWT_t = NW * T_t

    nc = bacc.Bacc("TRN2", target_bir_lowering=False, debug=False,
                   num_devices=NC)
    inp = {}

    def param(name, shape, dt):
        h = nc.declare_dram_parameter(name, list(shape), dt, isOutput=False)
        inp[name] = h
        return h

    # int8 node features in scaled units (scale folded into layer-0 W)
    param("tbl_t8", (SHARD_PAD, D), I8)
    param("tbl_s8", (SHARD_PAD, D), I8)
    for d, T in (("s", T_s), ("t", T_t)):
        param(f"idx_{d}", (16, NW * T * 8), I16)
        param(f"rel8_{d}", (P, NW * T), I8)
        param(f"wgt16_{d}", (P, NW * T), F16)
    # [iota|ident|s0Wl|s0Wr|t0Wl|t0Wr|s1Wl|s1Wr|t1Wl|t1Wr] (transposed W's)
    param("c16", (P, 10 * P), F16)
    param("c32", (P, 4), F32)                      # bias cols s0,t0,s1,t1

    outQ = nc.declare_dram_parameter("outQ", [2 * P, SHARD_PAD], I8,
                                     isOutput=True)
    outS = nc.declare_dram_parameter("outS", [2 * P, 1], F32, isOutput=True)

    rg = [list(range(NC))]

    with tile.TileContext(nc) as tc:
        with (
            tc.tile_pool(name="const", bufs=1) as cp,
            tc.tile_pool(name="mpool", bufs=3) as mp,
            tc.tile_pool(name="spool", bufs=2) as sp,
            tc.tile_pool(name="work", bufs=3) as wp,
            tc.tile_pool(name="xpool", bufs=2) as xp,
            tc.tile_pool(name="psA", bufs=2, space="PSUM") as pA,
            tc.tile_pool(name="psB", bufs=2, space="PSUM") as pB,
            tc.tile_pool(name="psT", bufs=2, space="PSUM") as pT,
            tc.tile_pool(name="dram", bufs=1, space="DRAM") as dp,
        ):
            def load(name):
                h = inp[name]
                t_ = cp.tile(list(h.shape), h.dtype, name=f"sb_{name}")
                nc.sync.dma_start(out=t_[:], in_=h[:])
                return t_

            c16_sb = load("c16")
            c32_sb = load("c32")
            iota_sb = c16_sb[:, 0:P]
            ident_sb = c16_sb[:, P:2 * P]
            W = {}
            for i, nm in enumerate(("s0", "t0", "s1", "t1")):
                W[f"{nm}_WlT"] = c16_sb[:, (2 + 2 * i) * P:(3 + 2 * i) * P]
                W[f"{nm}_WrT"] = c16_sb[:, (3 + 2 * i) * P:(4 + 2 * i) * P]
                W[f"{nm}_b"] = c32_sb[:, i:i + 1]

            meta = {}
            for d, T in (("s", T_s), ("t", T_t)):
                # replicate the 16-partition index stream to 128 partitions
                idx_sb = cp.tile([P, NW * T * 8], I16, name=f"sb_idx_{d}")
                for k in range(8):
                    nc.sync.dma_start(out=idx_sb[k * 16:(k + 1) * 16, :],
                                      in_=inp[f"idx_{d}"][:])
                # ship narrow, widen on device (tensor_scalar needs f32)
                r8 = load(f"rel8_{d}")
                w16 = load(f"wgt16_{d}")
                r32 = cp.tile([P, NW * T], F32, name=f"sb_rel32_{d}")
                w32 = cp.tile([P, NW * T], F32, name=f"sb_wgt32_{d}")
                nc.vector.tensor_copy(out=r32[:], in_=r8[:])
                nc.vector.tensor_copy(out=w32[:], in_=w16[:])
                meta[d] = (idx_sb, r32, w32)

            # Pre-touch DVE-read constants with tiny copies so the first
            # TensorScalarPtr doesn't need multiple DMA sem waits (ISA limit).
            for _i, _ap in enumerate(
                (c16_sb, meta["s"][1], meta["s"][2], meta["t"][1], meta["t"][2])
            ):
                warm = wp.tile([P, 1], F32, tag=f"warm{_i}", name=f"warm{_i}")
                nc.vector.reduce_sum(out=warm[:], in_=_ap[:],
                                     axis=mybir.AxisListType.X)

            # --- int8 shards -> fp16 DRAM tiles (+ transposed SBUF copies),
            #     then AllGather full fp16 gather tables ------------------
            tbl_t_full = dp.tile([N_PAD, D], F16, name="tbl_t_full",
                                 addr_space="Shared")
            tbl_s_full = dp.tile([N_PAD, D], F16, name="tbl_s_full",
                                 addr_space="Shared")
            t_sh_cp = dp.tile([SHARD_PAD, D], F16, name="t_sh_cp")
            s_sh_cp = dp.tile([SHARD_PAD, D], F16, name="s_sh_cp")
            tT_sb = cp.tile([P, SHARD_PAD], F16, name="tT_sb")
            sT_sb = cp.tile([P, SHARD_PAD], F16, name="sT_sb")

            for pnm, cp_tile, xT in (("tbl_t8", t_sh_cp, tT_sb),
                                     ("tbl_s8", s_sh_cp, sT_sb)):
                for wnd in range(NW):
                    r0 = wnd * P
                    raw = xp.tile([P, P], I8, tag="craw", name="craw")
                    nc.sync.dma_start(out=raw[:],
                                      in_=inp[pnm][r0:r0 + P, :])
                    cnv = xp.tile([P, P], F16, tag="cnv", name="cnv")
                    nc.vector.tensor_copy(out=cnv[:], in_=raw[:])
                    nc.sync.dma_start(
                        out=cp_tile[wnd * P:(wnd + 1) * P, :], in_=cnv[:])
                    tp = pT.tile([P, P], F16, tag="xt", name="xt_ps")
                    nc.tensor.transpose(tp[:], cnv[:], ident_sb)
                    nc.vector.tensor_copy(
                        out=xT[:, wnd * P:(wnd + 1) * P], in_=tp[:])

            nc.gpsimd.collective_compute(
                "AllGather", mybir.AluOpType.bypass, replica_groups=rg,
                ins=[s_sh_cp.opt()], outs=[tbl_s_full.opt()],
            )
            nc.gpsimd.collective_compute(
                "AllGather", mybir.AluOpType.bypass, replica_groups=rg,
                ins=[t_sh_cp.opt()], outs=[tbl_t_full.opt()],
            )

            s1T_sb = cp.tile([P, SHARD_PAD], F16, name="s1T_sb")
            t1T_sb = cp.tile([P, SHARD_PAD], F16, name="t1T_sb")

            t1_loc = dp.tile([SHARD_PAD, D], F16, name="t1_loc")
            s1_loc = dp.tile([SHARD_PAD, D], F16, name="s1_loc")

            s2T_sb = cp.tile([P, SHARD_PAD], F16, name="s2T_sb")
            t2T_sb = cp.tile([P, SHARD_PAD], F16, name="t2T_sb")

            def sage(T_lo, T_hi, mkey, table_ap, xT_sb, wpre,
                     storeT_sb=None, rows_dram=None, out_sb=None):
                """One direction of one SAGE layer over all windows.

                layer 0 (storeT_sb/rows_dram set): writes relu(out)^T into
                storeT_sb and relu(out) rows into rows_dram (via PE
                transpose of the column result).
                layer 1 (out_sb set): writes out^T into the SBUF buffer
                (quantized to int8 + shipped at the end).
                """
                T = T_lo + T_hi
                idx_sb, rel_sb, wgt_sb = meta[mkey]
                WlT = W[f"{wpre}_WlT"]
                WrT = W[f"{wpre}_WrT"]
                bcol = W[f"{wpre}_b"]
                tbl_rows = table_ap.shape[0]
                for wnd in range(NW):
                    nsl = slice(wnd * P, (wnd + 1) * P)
                    msg = mp.tile([P, T * P], F16, tag="msg", name="msg")
                    ib = wnd * T * 8
                    if T_lo > 0:
                        nc.gpsimd.dma_gather(
                            out_ap=msg[:, 0:T_lo * P].rearrange(
                                "p (c e) -> p c e", e=P),
                            in_ap=table_ap[0:min(HALF, tbl_rows), :],
                            idxs_ap=idx_sb[:, ib:ib + T_lo * 8],
                            num_idxs=T_lo * P,
                            num_idxs_reg=T_lo * P,
                            elem_size=P,
                            single_packet=False,
                        )
                    if T_hi > 0:
                        nc.gpsimd.dma_gather(
                            out_ap=msg[:, T_lo * P:T * P].rearrange(
                                "p (c e) -> p c e", e=P),
                            in_ap=table_ap[HALF:tbl_rows, :],
                            idxs_ap=idx_sb[:, ib + T_lo * 8:ib + T * 8],
                            num_idxs=T_hi * P,
                            num_idxs_reg=T_hi * P,
                            elem_size=P,
                            single_packet=False,
                        )
                    agg_ps = pA.tile([P, P], F32, tag="agg", name="agg_ps")
                    # One big selection tile per window; the leading memset
                    # absorbs slot-recycle waits so each TensorScalarPtr
                    # carries at most one (ISA sync-slot limit).
                    sel_big = sp.tile([P, T * P], F16, tag="selbig",
                                      name="sel_big")
                    nc.vector.memset(sel_big[:], 0)
                    for c in range(T):
                        col = wnd * T + c
                        sel = sel_big[:, c * P:(c + 1) * P]
                        nc.vector.tensor_scalar(
                            out=sel,
                            in0=iota_sb,
                            scalar1=rel_sb[:, col:col + 1],
                            scalar2=wgt_sb[:, col:col + 1],
                            op0=mybir.AluOpType.is_equal,
                            op1=mybir.AluOpType.mult,
                        )
                        nc.tensor.matmul(
                            out=agg_ps[:],
                            lhsT=msg[:, c * P:(c + 1) * P],
                            rhs=sel,
                            start=(c == 0),
                            stop=(c == T - 1),
                        )
                    a_sb = wp.tile([P, P], F16, tag="a", name="a_sb")
                    nc.vector.tensor_copy(out=a_sb[:], in_=agg_ps[:])

                    o1 = pB.tile([P, P], F32, tag="o1", name="o1")
                    nc.tensor.matmul(out=o1[:], lhsT=WlT, rhs=a_sb[:],
                                     start=True, stop=False)
                    nc.tensor.matmul(out=o1[:], lhsT=WrT, rhs=xT_sb[:, nsl],
                                     start=False, stop=True)
                    if storeT_sb is not None:
                        nc.scalar.activation(
                            out=storeT_sb[:, nsl], in_=o1[:],
                            func=mybir.ActivationFunctionType.Relu,
                            bias=bcol,
                        )
                        # row orientation = PE transpose of the column result
                        rp = pT.tile([P, P], F16, tag="xt", name="row_ps")
                        nc.tensor.transpose(rp[:], storeT_sb[:, nsl],
                                            ident_sb)
                        r16 = wp.tile([P, P], F16, tag="r16", name="r16")
                        nc.vector.tensor_copy(out=r16[:], in_=rp[:])
                        nc.sync.dma_start(out=rows_dram[nsl, :], in_=r16[:])
                    else:
                        nc.scalar.activation(
                            out=out_sb[:, nsl], in_=o1[:],
                            func=mybir.ActivationFunctionType.Identity,
                            bias=bcol,
                        )

            t1_full = dp.tile([N_PAD, D], F16, name="t1_full",
                              addr_space="Shared")
            s1_full = dp.tile([N_PAD, D], F16, name="s1_full",
                              addr_space="Shared")
            # layer 0, t-direction: t1 = relu(sage over flipped edges of s)
            sage(Tlo_t, Thi_t, "t", tbl_s_full[:], sT_sb, "t0",
                 storeT_sb=t1T_sb, rows_dram=t1_loc)
            nc.gpsimd.collective_compute(
                "AllGather", mybir.AluOpType.bypass, replica_groups=rg,
                ins=[t1_loc.opt()], outs=[t1_full.opt()],
            )
            # layer 0, s-direction: s1
            sage(Tlo_s, Thi_s, "s", tbl_t_full[:], tT_sb, "s0",
                 storeT_sb=s1T_sb, rows_dram=s1_loc)
            nc.gpsimd.collective_compute(
                "AllGather", mybir.AluOpType.bypass, replica_groups=rg,
                ins=[s1_loc.opt()], outs=[s1_full.opt()],
            )
            # layer 1
            sage(Tlo_s, Thi_s, "s", t1_full[:], t1T_sb, "s1", out_sb=s2T_sb)
            sage(Tlo_t, Thi_t, "t", s1_full[:], s1T_sb, "t1", out_sb=t2T_sb)

            # --- int8-quantize outputs with per-partition (feature) scales
            for row0, buf in ((0, s2T_sb), (P, t2T_sb)):
                amax = wp.tile([P, 1], F32, tag="amax", name="amax")
                nc.vector.tensor_reduce(
                    out=amax[:], in_=buf[:], op=mybir.AluOpType.max,
                    axis=mybir.AxisListType.X, apply_absolute_value=True)
                nc.vector.tensor_scalar_max(out=amax[:], in0=amax[:],
                                            scalar1=1e-6)
                qs = wp.tile([P, 1], F32, tag="qs", name="qs")
                nc.vector.reciprocal(out=qs[:], in_=amax[:])
                nc.vector.tensor_scalar(
                    out=qs[:], in0=qs[:], scalar1=127.0, scalar2=None,
                    op0=mybir.AluOpType.mult)
                qi8 = wp.tile([P, SHARD_PAD], I8, tag="qi8", name="qi8")
                nc.vector.tensor_scalar(
                    out=qi8[:], in0=buf[:], scalar1=qs[:, 0:1], scalar2=None,
                    op0=mybir.AluOpType.mult)
                nc.sync.dma_start(out=outQ[row0:row0 + P, :], in_=qi8[:])
                scol = wp.tile([P, 1], F32, tag="scol", name="scol")
                nc.vector.tensor_scalar(
                    out=scol[:], in0=amax[:], scalar1=1.0 / 127.0,
                    scalar2=None, op0=mybir.AluOpType.mult)
                nc.sync.dma_start(out=outS[row0:row0 + P, :], in_=scol[:])

    nc.compile()
    return nc


# ---------------------------------------------------------------------------
# Cached jit runner (replicates run_bass_kernel_spmd's axon path, but with a
# persistent jit closure, committed device inputs, and on-device zeros)
# ---------------------------------------------------------------------------

_RUNNER_CACHE = {}


def _get_runner(nc, NC):
    key = id(nc)
    if key in _RUNNER_CACHE:
        return _RUNNER_CACHE[key]

    import jax
    import jax.numpy as jnp
    from jax.sharding import Mesh, PartitionSpec, NamedSharding
    from jax.experimental.shard_map import shard_map
    from concourse import bass2jax

    bass2jax.install_neuronx_cc_hook()
    partition_name = (nc.partition_id_tensor.name
                      if nc.partition_id_tensor else None)
    in_names, out_names, out_avals = [], [], []
    for alloc in nc.m.functions[0].allocations:
        if not isinstance(alloc, mybir.MemoryLocationSet):
            continue
        name = alloc.memorylocations[0].name
        if alloc.kind == "ExternalInput":
            if name != partition_name:
                in_names.append(name)
        elif alloc.kind == "ExternalOutput":
            shape = tuple(alloc.tensor_shape)
            dtype = mybir.dt.np(alloc.dtype)
            out_names.append(name)
            out_avals.append(jax.core.ShapedArray(shape, dtype))
    n_params = len(in_names)
    n_outs = len(out_avals)
    in_names_all = (in_names + out_names
                    + ([partition_name] if partition_name else []))

    def _body(*args):
        operands = list(args)
        if partition_name is not None:
            operands.append(bass2jax.partition_id_tensor())
        outs = bass2jax._bass_exec_p.bind(
            *operands, out_avals=tuple(out_avals),
            in_names=tuple(in_names_all), out_names=tuple(out_names),
            lowering_input_output_aliases=(),
            sim_require_finite=True, sim_require_nnan=True, nc=nc)
        return tuple(outs)

    devices = jax.devices()[:NC]
    mesh = Mesh(np.asarray(devices), ("core",))
    ns = NamedSharding(mesh, PartitionSpec("core"))
    in_specs = (PartitionSpec("core"),) * (n_params + n_outs)
    out_specs = (PartitionSpec("core"),) * n_outs
    donate = tuple(range(n_params, n_params + n_outs))
    sharded = jax.jit(
        shard_map(_body, mesh=mesh, in_specs=in_specs, out_specs=out_specs,
                  check_rep=False),
        donate_argnums=donate, keep_unused=True)

    zshapes = [(NC * a.shape[0], *a.shape[1:]) for a in out_avals]
    zdts = [a.dtype for a in out_avals]
    zfun = jax.jit(
        lambda: tuple(jnp.zeros(sh, dt) for sh, dt in zip(zshapes, zdts)),
        out_shardings=tuple([ns] * n_outs))

    dbg_name = nc.dbg_addr.name if nc.dbg_addr is not None else None
    runner = (sharded, zfun, in_names, out_names, ns, dbg_name)
    _RUNNER_CACHE[key] = runner
    return runner


# ---------------------------------------------------------------------------
# Full pipeline
# ---------------------------------------------------------------------------

_PROGRAM_CACHE = {}
LAST_RUN = None

_FETCH_POOL = None


def _fetch_pool():
    global _FETCH_POOL
    if _FETCH_POOL is None:
        import concurrent.futures as cf
        _FETCH_POOL = cf.ThreadPoolExecutor(8)
    return _FETCH_POOL


def _put_sharded(a, NC, ns, devices, replicate=False):
    """Threaded per-device put: transfers the 8 shards in parallel streams."""
    import jax

    pool = _fetch_pool()
    if replicate:
        pieces = [a] * NC
        gshape = (NC * a.shape[0], *a.shape[1:])
    else:
        pieces = a.reshape(NC, a.shape[0] // NC, *a.shape[1:])
        gshape = a.shape
    futs = [pool.submit(jax.device_put, pieces[i], devices[i])
            for i in range(NC)]
    arrs = [f.result() for f in futs]
    return jax.make_array_from_single_device_arrays(gshape, ns, arrs)


def kernel(s, t, edge_index, edge_weight, **wdict):
    import jax
    import os
    import time

    _tv = os.environ.get("BASS_GNN_T", "") == "1"
    _t0 = time.time()

    def _tick(msg):
        if _tv:
            print(f"  [kernel] {msg}: {time.time() - _t0:.3f}s", flush=True)

    N = s.shape[0]
    NC = 8
    SHARD = N // NC
    NW = -(-SHARD // P)
    SHARD_PAD = NW * P

    s = np.asarray(s, dtype=np.float32)
    t = np.asarray(t, dtype=np.float32)

    # --- stage 1: int8 feature shards (cheap) -> start uploading now ---
    # clip at 3.9 sigma: MSE-optimal int8 quantization for ~gaussian data
    dev = {}
    ns_hint = _RUNNER_CACHE.get("ns_hint")
    if ns_hint is None:
        from jax.sharding import Mesh, PartitionSpec, NamedSharding
        mesh = Mesh(np.asarray(jax.devices()[:NC]), ("core",))
        ns_hint = NamedSharding(mesh, PartitionSpec("core"))
        _RUNNER_CACHE["ns_hint"] = ns_hint
    devs = list(ns_hint.mesh.devices.reshape(-1))

    qbuf = np.empty((N, D), np.float32)

    def q8tbl(x, sc):
        np.multiply(x, 1.0 / sc, out=qbuf)
        np.rint(qbuf, out=qbuf)
        np.clip(qbuf, -127, 127, out=qbuf)
        out = np.zeros((NC, SHARD_PAD, D), np.int8)
        out[:, :SHARD] = qbuf.astype(np.int8).reshape(NC, SHARD, D)
        return out.reshape(NC * SHARD_PAD, D)

    sc_t = min(float(np.abs(t).max()), 3.9 * float(t.std())) / 127.0
    dev["tbl_t8"] = _put_sharded(q8tbl(t, sc_t), NC, ns_hint, devs)
    sc_s = min(float(np.abs(s).max()), 3.9 * float(s.std())) / 127.0
    dev["tbl_s8"] = _put_sharded(q8tbl(s, sc_s), NC, ns_hint, devs)
    _tick("tables built+put dispatched")

    # --- stage 2: edge preprocessing (sort-heavy) ---
    src = np.asarray(edge_index[0], dtype=np.int32)
    dst = np.asarray(edge_index[1], dtype=np.int32)
    ew = np.asarray(edge_weight, dtype=np.float32)

    w = (1.0 / (1.0 + np.exp(-ew))).astype(np.float32)
    deg_in = np.bincount(dst, minlength=N).astype(np.float32)
    deg_out = np.bincount(src, minlength=N).astype(np.float32)
    inv_in = (1.0 / np.maximum(deg_in, 1.0)).astype(np.float32)
    inv_out = (1.0 / np.maximum(deg_out, 1.0)).astype(np.float32)

    # s-updates aggregate over dst (gather src); t-updates aggregate over src
    Tlo_s, Thi_s, idx_s, rel_s, wgt_s = _prep_direction(
        dst, src, w * inv_in[dst], N, NC)
    dev["idx_s"] = _put_sharded(idx_s, NC, ns_hint, devs)
    dev["rel8_s"] = _put_sharded(rel_s, NC, ns_hint, devs)
    dev["wgt16_s"] = _put_sharded(wgt_s, NC, ns_hint, devs)
    Tlo_t, Thi_t, idx_t, rel_t, wgt_t = _prep_direction(
        src, dst, w * inv_out[src], N, NC)
    dev["idx_t"] = _put_sharded(idx_t, NC, ns_hint, devs)
    dev["rel8_t"] = _put_sharded(rel_t, NC, ns_hint, devs)
    dev["wgt16_t"] = _put_sharded(wgt_t, NC, ns_hint, devs)
    _tick("edge prep + meta puts dispatched")

    # --- stage 3: small constants (scales folded into layer-0 weights) ---
    iota = np.broadcast_to(np.arange(P, dtype=np.float16), (P, P))
    c16 = [np.ascontiguousarray(iota), np.eye(P, dtype=np.float16)]
    c32 = np.empty((P, 4), np.float32)
    for i, nm in enumerate(("s0", "t0", "s1", "t1")):
        Wl = np.asarray(wdict[f"{nm}_Wl"], np.float32)
        Wr = np.asarray(wdict[f"{nm}_Wr"], np.float32)
        sc = {"s0": sc_t, "t0": sc_s}.get(nm, 1.0)
        c16.append((Wl.T * sc).astype(np.float16))
        c16.append((Wr.T * sc).astype(np.float16))
        c32[:, i] = np.asarray(wdict[f"{nm}_bl"], np.float32)

    dev["c16"] = _put_sharded(np.concatenate(c16, axis=1), NC, ns_hint,
                              devs, replicate=True)
    dev["c32"] = _put_sharded(c32, NC, ns_hint, devs, replicate=True)
    _tick("consts put dispatched")

    # --- program + runner ---
    key = (N, NC, Tlo_s, Thi_s, Tlo_t, Thi_t)
    if key not in _PROGRAM_CACHE:
        _PROGRAM_CACHE[key] = build_program(N, NC, Tlo_s, Thi_s, Tlo_t, Thi_t)
    nc = _PROGRAM_CACHE[key]
    sharded, zfun, in_names, out_names, ns, dbg_name = _get_runner(nc, NC)
    _tick("runner ready")

    if dbg_name is not None and dbg_name not in dev:
        dev[dbg_name] = jax.device_put(
            np.zeros((NC * 1, 2), np.uint32), ns)

    dz = zfun()
    outs = sharded(*[dev[nm] for nm in in_names], *dz)
    oq = outs[out_names.index("outQ")]
    osc = outs[out_names.index("outS")]
    oq.block_until_ready()
    _tick("exec done (incl input upload wait)")
    osc.copy_to_host_async()
    pool = _fetch_pool()
    parts = [None] * NC

    def _fetch(sh):
        parts[sh.index[0].start // (2 * P)] = np.asarray(sh.data)

    list(pool.map(_fetch, oq.addressable_shards))
    got_s = np.asarray(osc)
    _tick("D2H done")

    # --- assemble: int8 [NC, 2P, SHARD_PAD] x per-feature scales -> f32 ---
    gq = np.stack(parts, axis=0)
    gs = got_s.reshape(NC, 2 * P, 1)

    def unshard(q, sc):
        # q [NC, P, SHARD_PAD] i8, sc [NC, P, 1] f32 -> [N, D] f32
        return (np.ascontiguousarray(
            q[:, :, :SHARD].transpose(0, 2, 1)).astype(np.float32)
            * sc.transpose(0, 2, 1)).reshape(N, D)

    return (unshard(gq[:, :P], gs[:, :P]),
            unshard(gq[:, P:], gs[:, P:]))
